# revision 11
# baseline (speedup 1.0000x reference)
"""Trainium2 Bass kernel for BioNet message-passing recurrence.

Reference computes 50 steps of Jacobi iteration  X <- mml(W @ X + X_bias)
with W (8192x8192 f32, masked) and X (8192x32), returning X.T (32, 8192).
The iteration is a contraction converging to a fixed point; ANY update
schedule converging to the same fixed point gives the same answer.  The
kernel uses block GAUSS-SEIDEL over two global half-blocks (nodes
[1024c,1024c+512) = half A, rest = half B): updating one half per
"half-step" using the freshest available other half converges in 6
sweeps (12 half-steps, verified to the bf16 noise floor) vs 11+ Jacobi
steps, and needs only ONE AllGather per half-step — which matters
because each collective costs ~6-8us of mostly-fixed protocol latency
on the single CC core.

Per-core layout (8 NeuronCores, tensor-parallel over W rows):
  - A tiny warmup AllGather is dispatched first so the one-time NRT
    comm-init / core start-skew cost (~70us) overlaps the W load.
  - Each core holds rows [1024c, 1024c+1024) of W transposed in SBUF as
    bf16 (16.8 MB/core); DMAed in 8 K-chunks so step 2 starts early.
  - Half-step updating half U: 16 quads of out^T = X^T @ W^T with X
    (128,32) tiles stationary, 4-way column-tiled (tile_position) for
    ~4x PE throughput at batch=32.  Quads over U-parity K-columns use
    the 2-half-steps-old U state (long available, pre-run during the
    previous collective's flight); quads over the other parity wait for
    the just-gathered fresh half.
  - The 4 column-group partials land on partition groups 32j..32j+32 of
    PSUM; a small PE pass with selector S[p,b] = (p%32==b) fuses the
    4-way reduction with the (batch,node)->(node,batch) transpose; the
    PSUM->SBUF copy is split 4-ways to pipeline with the S-mms.
  - Activation uses  mml(z) = min(LeakyRelu_leak(z), 1-0.25/max(z,0.5))
    (algebraically exact); the LeakyRelu branch runs on GPSIMD in
    parallel with the saturation branch on DVE.
  - Queues: staging SBUF->DRAM and unload DMAs on sync (the unload's
    collective wait parks after the staging it follows, blocking
    nothing), collective triggers on gpsimd, LeakyRelu on the scalar
    queue, so no critical op ever queues behind a collective wait.
  - Paced dummy-matmul bursts keep the PE from idling >3.4us during the
    collective window, which would trip HAM clock-throttling (halves
    the PE clock).
"""

import os
import sys
import types

sys.path.insert(0, "/opt/trn_rl_repo")

import numpy as np
import ml_dtypes

import concourse.bass as bass
import concourse.mybir as mybir
import concourse.tile as tile
from concourse import bacc
import concourse.bass_utils as bass_utils
from concourse.bass import ts
from concourse.bass_utils import run_bass_kernel_spmd

N_NODES = 8192
N_CORES = 8
BATCH = 32
GS_HALF_STEPS = 13                  # 6.5 Gauss-Seidel sweeps (see module doc)
LEAK = 0.01
LOCAL = N_NODES // N_CORES          # 1024 rows per core
K_TILES = N_NODES // 128            # 64
LOCAL_TILES = LOCAL // 128          # 8
CHUNK_F = LOCAL_TILES * BATCH       # 256 free elems per activated chunk
HALF_F = CHUNK_F // 2               # 128
N_QUADS = K_TILES // 4              # 16
EVENS = list(range(0, N_QUADS, 2))  # K-columns of the A halves
ODDS = list(range(1, N_QUADS, 2))   # K-columns of the B halves

LAST_RESULTS = None  # BassKernelResults of the most recent run (for test.py)


def setup_tracing():
    """Register the axon NTFF profile hook; the container's antenv is a stub."""
    try:
        import antenv
        if "antenv.axon_hooks" not in sys.modules:
            mod = types.ModuleType("antenv.axon_hooks")
            mod._hook = None
            mod.set_axon_ntff_profile_hook = lambda h: setattr(mod, "_hook", h)
            mod.get_axon_ntff_profile_hook = lambda: mod._hook
            sys.modules["antenv.axon_hooks"] = mod
            antenv.axon_hooks = mod
            from trn_agent_boot.trn_boot import _ntff_profile_via_ctypes
            mod.set_axon_ntff_profile_hook(
                _ntff_profile_via_ctypes("/opt/axon/libaxon_pjrt.so")
            )
        bass_utils.upload_artifacts = lambda tmpdir: f"local://{tmpdir}"
    except Exception:
        pass


def build_nc():
    nc = bacc.Bacc(None, target_bir_lowering=False, num_devices=N_CORES)
    f32 = mybir.dt.float32
    bf16 = mybir.dt.bfloat16
    warm_bursts = int(os.environ.get("WARM_BURSTS", "7"))
    warm_per = int(os.environ.get("WARM_PER", "3"))
    pace_cols = int(os.environ.get("PACE_COLS", "4096"))

    # Per-core inputs (shapes identical on every core; contents sharded).
    wt = nc.dram_tensor("wt", [N_NODES, LOCAL], bf16, kind="ExternalInput")
    xb = nc.dram_tensor("xb", [128, CHUNK_F], f32, kind="ExternalInput")
    s_in = nc.dram_tensor("s_in", [128, BATCH], bf16, kind="ExternalInput")
    out = nc.dram_tensor("out", [128, CHUNK_F], f32, kind="ExternalOutput")

    with tile.TileContext(nc) as tc:
        with (
            tc.tile_pool(name="persist", bufs=1) as persist,
            tc.tile_pool(name="ys", bufs=2) as ys_pool,
            tc.tile_pool(name="chain", bufs=2) as chain,
            tc.tile_pool(name="stage", bufs=2) as stage_pool,
            tc.tile_pool(name="psum", bufs=2, space="PSUM") as psum_pool,
            tc.tile_pool(name="psumt", bufs=2, space="PSUM") as psumt_pool,
            tc.tile_pool(name="dram", bufs=2, space="DRAM") as dram,
        ):
            # ---- comm warmup: absorb NRT comm-init + core start skew ---
            warm_in = dram.tile([128, 1], bf16, tag="wi", name="warm_in",
                                bufs=1)
            warm_out = dram.tile([128 * N_CORES, 1], bf16, addr_space="Shared",
                                 tag="wo", name="warm_out", bufs=1)
            nc.gpsimd.collective_compute(
                "AllGather",
                mybir.AluOpType.bypass,
                replica_groups=[list(range(N_CORES))],
                ins=[warm_in.opt()],
                outs=[warm_out.opt()],
            )

            # ---- persistent SBUF tensors -------------------------------
            xb_sb = persist.tile([128, CHUNK_F], f32)
            nc.sync.dma_start(out=xb_sb, in_=xb[:])
            s_sb = persist.tile([128, BATCH], bf16)
            nc.scalar.dma_start(out=s_sb, in_=s_in[:])
            wt_sb = persist.tile([128, K_TILES, LOCAL], bf16)      # 128 KB/part
            wt_v = wt.rearrange("(t p) n -> p t n", p=128)
            for i in range(8):
                eng = nc.sync if i % 2 == 0 else nc.scalar
                eng.dma_start(
                    out=wt_sb[:, 8 * i : 8 * (i + 1), :],
                    in_=wt_v[:, 8 * i : 8 * (i + 1), :],
                )
            x_buf = persist.tile([128, K_TILES * BATCH], bf16)
            out_f32 = persist.tile([128, CHUNK_F], f32)
            pw_a = persist.tile([128, pace_cols], f32, name="pw_a")
            pw_b = persist.tile([128, pace_cols], f32, name="pw_b")
            nc.vector.memset(pw_a, 0.0)
            nc.vector.memset(pw_b, 0.0)

            def activation(z_src, to_bf, also_f32=None, width=HALF_F):
                """to_bf[:] = mml(z_src); optionally also f32 copy.

                mml(z) = min(LeakyRelu_leak(z), 1 - 0.25/max(z, 0.5))
                (exact for |z| < ~99, which holds here).  LeakyRelu branch
                on GPSIMD overlaps the DVE saturation-branch chain.
                """
                lr_t = chain.tile([128, width], f32, tag="lr", name="lr_t")
                nc.scalar.activation(
                    lr_t, z_src, mybir.ActivationFunctionType.Lrelu,
                    alpha=LEAK,
                )
                m_t = chain.tile([128, width], f32, tag="m", name="m_t")
                nc.vector.tensor_scalar_max(m_t, z_src, 0.5)
                r_t = chain.tile([128, width], f32, tag="r", name="r_t")
                nc.vector.reciprocal_approx_fast(out=r_t, in_=m_t)
                s_t = chain.tile([128, width], f32, tag="s", name="s_t")
                nc.vector.tensor_scalar(
                    s_t, r_t, -0.25, 1.0,
                    mybir.AluOpType.mult, mybir.AluOpType.add,
                )
                nc.vector.tensor_tensor(to_bf, lr_t, s_t, mybir.AluOpType.min)
                if also_f32 is not None:
                    nc.vector.tensor_tensor(
                        also_f32, lr_t, s_t, mybir.AluOpType.min
                    )

            def tail_half(psum_hv, v, write_out):
                """Reduce+transpose (S-matrix PE pass), bias+activation for
                output half v; returns the staged bf16 (128, HALF_F) tile."""
                psum_t = psumt_pool.tile(
                    [128, HALF_F], mybir.dt.float32, tag="pt", name="psum_t"
                )
                for tt_ in range(4):
                    ysb = ys_pool.tile(
                        [128, 128], bf16, tag=f"ys{tt_}", name=f"ysb{tt_}"
                    )
                    nc.vector.tensor_copy(ysb, psum_hv[:, ts(tt_, 128)])
                    nc.tensor.matmul(
                        psum_t[:, ts(tt_, BATCH)],
                        ysb,
                        s_sb,
                        start=True,
                        stop=True,
                    )
                hs = ts(v, HALF_F)
                z_t = chain.tile([128, HALF_F], mybir.dt.float32,
                                 tag="z", name="z_t")
                nc.vector.tensor_tensor(
                    z_t, psum_t, xb_sb[:, hs], mybir.AluOpType.add
                )
                stage_v = stage_pool.tile(
                    [128, HALF_F], bf16, tag=f"st{v}", name=f"stage{v}"
                )
                activation(
                    z_t,
                    stage_v,
                    also_f32=out_f32[:, hs] if write_out else None,
                )
                return stage_v

            def broadcast_half(stage_v, v):
                """AllGather one staged half into x_buf's half-v columns."""
                ag_in = dram.tile([128, HALF_F], bf16, tag=f"agi{v}",
                                  name=f"ag_in{v}")
                nc.sync.dma_start(out=ag_in, in_=stage_v)
                ag_out = dram.tile(
                    [128 * N_CORES, HALF_F], bf16, addr_space="Shared",
                    tag=f"ago{v}", name=f"ag_out{v}",
                )
                nc.gpsimd.collective_compute(
                    "AllGather",
                    mybir.AluOpType.bypass,
                    replica_groups=[list(range(N_CORES))],
                    ins=[ag_in.opt()],
                    outs=[ag_out.opt()],
                )
                # strided unload: chunk c -> x_buf cols [256c+128v, +128);
                # split 2-way so the first fresh quads' data lands sooner
                dst_v = x_buf.rearrange("p (c f) -> p c f", c=N_CORES)[
                    :, :, HALF_F * v : HALF_F * (v + 1)
                ]
                src_v = ag_out.rearrange("(c p) f -> p c f", p=128)
                nc.sync.dma_start(out=dst_v[:, 0:2], in_=src_v[:, 0:2])
                nc.sync.dma_start(out=dst_v[:, 2:], in_=src_v[:, 2:])

            def pe_warm():
                """Paced dummy matmuls through the collective window so HAM
                never sees a >3.4us PE idle gap (which halves the clock)."""
                if warm_bursts <= 0:
                    return
                psum_w = psumt_pool.tile(
                    [128, 512], mybir.dt.float32, tag="pw", name="psum_w",
                    bufs=1,
                )

                def burst(dep):
                    for _ in range(warm_per):
                        wmm = nc.tensor.matmul(
                            psum_w[0:BATCH, 0:128], s_sb, wt_sb[:, 0, 0:128],
                            start=True, stop=True, skip_group_check=True,
                        )
                        if dep is not None:
                            bass._add_dep_helper(
                                wmm.ins, dep.ins, True, "pace warm mm"
                            )

                burst(None)
                for i in range(warm_bursts):
                    src, dst = (pw_a, pw_b) if i % 2 == 0 else (pw_b, pw_a)
                    cp = nc.vector.tensor_copy(dst, src)
                    burst(cp)

            def mm_quads(h, psum_hv, quads, start, stop):
                for qi, q in enumerate(quads):
                    for j in range(4):
                        k = 4 * q + j
                        nc.tensor.matmul(
                            psum_hv[32 * j : 32 * (j + 1), :],
                            x_buf[:, ts(k, BATCH)],
                            wt_sb[:, k, ts(h, 512)],
                            start=start and qi == 0,
                            stop=stop and qi == len(quads) - 1,
                            tile_position=(0, 32 * j),
                        )

            # ---- Gauss-Seidel half-steps -------------------------------
            # t odd: update half A (h=0);  t even: update half B (h=1).
            # Half-step t uses the fresh other half (gathered at t-1) and
            # its own 2-old half (gathered at t-2).
            for t in range(1, GS_HALF_STEPS + 1):
                h = (t + 1) % 2
                last = t == GS_HALF_STEPS
                write_out = t >= GS_HALF_STEPS - 1
                if t == 1:
                    # A(1) = mml(xb_A): state is zero, no matmuls
                    stage_v = stage_pool.tile(
                        [128, HALF_F], bf16, tag="st0", name="stage0"
                    )
                    activation(xb_sb[:, ts(0, HALF_F)], stage_v,
                               also_f32=out_f32[:, ts(0, HALF_F)]
                               if write_out else None)
                else:
                    stale = EVENS if h == 0 else ODDS
                    fresh = ODDS if h == 0 else EVENS
                    psum_hv = psum_pool.tile(
                        [128, 512], mybir.dt.float32, tag="pm", name="psum_m"
                    )
                    if t == 2:
                        # B(2) = mml(W_BA A(1) + xb_B): only A-columns
                        pe_warm()
                        mm_quads(1, psum_hv, EVENS, start=True, stop=True)
                    else:
                        mm_quads(h, psum_hv, stale, start=True, stop=False)
                        # paced warm fills the PE gap while the fresh
                        # half's collective is still in flight
                        pe_warm()
                        mm_quads(h, psum_hv, fresh, start=False, stop=True)
                    stage_v = tail_half(psum_hv, h, write_out)
                if last:
                    nc.sync.dma_start(out=out[:], in_=out_f32)
                else:
                    broadcast_half(stage_v, h)

    nc.compile()
    return nc


def _prepare_in_maps(X_full, weights, bias, edge_mask):
    W = np.where(edge_mask, weights, 0.0).astype(np.float32)
    Xb = X_full.astype(np.float32).T + bias.astype(np.float32)  # (n, B)
    S = np.zeros((128, BATCH), np.float32)
    S[np.arange(128), np.arange(128) % BATCH] = 1.0
    S = S.astype(ml_dtypes.bfloat16)
    in_maps = []
    for c in range(N_CORES):
        rows = slice(LOCAL * c, LOCAL * (c + 1))
        wt_c = np.ascontiguousarray(W[rows, :].T).astype(ml_dtypes.bfloat16)
        xb_c = (
            Xb[rows]                       # (1024, 32)
            .reshape(LOCAL_TILES, 128, BATCH)
            .transpose(1, 0, 2)
            .reshape(128, CHUNK_F)
            .copy()
        )
        in_maps.append({"wt": wt_c, "xb": xb_c, "s_in": S})
    return in_maps


def _reassemble(results):
    out = np.empty((BATCH, N_NODES), np.float32)
    for c in range(N_CORES):
        oc = np.asarray(results[c]["out"])  # (128, 256)
        chunk = (
            oc.reshape(128, LOCAL_TILES, BATCH)
            .transpose(1, 0, 2)
            .reshape(LOCAL, BATCH)
        )
        out[:, LOCAL * c : LOCAL * (c + 1)] = chunk.T
    return out


def kernel(X_full, weights, bias, edge_mask):
    global LAST_RESULTS
    setup_tracing()
    in_maps = _prepare_in_maps(X_full, weights, bias, edge_mask)
    nc = build_nc()
    res = run_bass_kernel_spmd(nc, in_maps, core_ids=list(range(N_CORES)))
    LAST_RESULTS = res
    return _reassemble(res.results)


if __name__ == "__main__":
    # quick self-run with random data
    rng = np.random.default_rng(0)
    X_full = rng.random((BATCH, N_NODES), np.float32)
    weights = rng.standard_normal((N_NODES, N_NODES), np.float32)
    bias = 0.001 * np.ones((N_NODES, 1), np.float32)
    edge_mask = rng.random((N_NODES, N_NODES)) < 0.002
    out = kernel(X_full, weights, bias, edge_mask)
    print("out", out.shape, out.dtype, out[:2, :4])


# revision 12
# speedup vs baseline: 1.1073x; 1.1073x over previous
"""Trainium2 Bass kernel for BioNet message-passing recurrence.

Reference computes 50 steps of Jacobi iteration  X <- mml(W @ X + X_bias)
with W (8192x8192 f32, masked) and X (8192x32), returning X.T (32, 8192).
The iteration is a contraction converging to a fixed point; ANY update
schedule converging to the same fixed point gives the same answer.  The
kernel uses block GAUSS-SEIDEL over two global half-blocks (nodes
[1024c,1024c+512) = half A, rest = half B): updating one half per
"half-step" using the freshest available other half converges in 6
sweeps (12 half-steps, verified to the bf16 noise floor) vs 11+ Jacobi
steps, and needs only ONE AllGather per half-step — which matters
because each collective costs ~6-8us of mostly-fixed protocol latency
on the single CC core.

Per-core layout (8 NeuronCores, tensor-parallel over W rows):
  - A tiny warmup AllGather is dispatched first so the one-time NRT
    comm-init / core start-skew cost (~70us) overlaps the W load.
  - Each core holds rows [1024c, 1024c+1024) of W transposed in SBUF as
    bf16 (16.8 MB/core); DMAed in 8 K-chunks so step 2 starts early.
  - Half-step updating half U: 16 quads of out^T = X^T @ W^T with X
    (128,32) tiles stationary, 4-way column-tiled (tile_position) for
    ~4x PE throughput at batch=32.  Quads over U-parity K-columns use
    the 2-half-steps-old U state (long available, pre-run during the
    previous collective's flight); quads over the other parity wait for
    the just-gathered fresh half.
  - The 4 column-group partials land on partition groups 32j..32j+32 of
    PSUM; a small PE pass with selector S[p,b] = (p%32==b) fuses the
    4-way reduction with the (batch,node)->(node,batch) transpose; the
    PSUM->SBUF copy is split 4-ways to pipeline with the S-mms.
  - Activation uses  mml(z) = min(LeakyRelu_leak(z), 1-0.25/max(z,0.5))
    (algebraically exact); the LeakyRelu branch runs on GPSIMD in
    parallel with the saturation branch on DVE.
  - Queues: staging SBUF->DRAM and unload DMAs on sync (the unload's
    collective wait parks after the staging it follows, blocking
    nothing), collective triggers on gpsimd, LeakyRelu on the scalar
    queue, so no critical op ever queues behind a collective wait.
  - Paced dummy-matmul bursts keep the PE from idling >3.4us during the
    collective window, which would trip HAM clock-throttling (halves
    the PE clock).
"""

import os
import sys
import types

sys.path.insert(0, "/opt/trn_rl_repo")

import numpy as np
import ml_dtypes

import concourse.bass as bass
import concourse.mybir as mybir
import concourse.tile as tile
from concourse import bacc
import concourse.bass_utils as bass_utils
from concourse.bass import ts
from concourse.bass_utils import run_bass_kernel_spmd

N_NODES = 8192
N_CORES = 8
BATCH = 32
GS_HALF_STEPS = 13                  # 6.5 Gauss-Seidel sweeps (see module doc)
LEAK = 0.01
LOCAL = N_NODES // N_CORES          # 1024 rows per core
K_TILES = N_NODES // 128            # 64
LOCAL_TILES = LOCAL // 128          # 8
CHUNK_F = LOCAL_TILES * BATCH       # 256 free elems per activated chunk
HALF_F = CHUNK_F // 2               # 128
N_QUADS = K_TILES // 4              # 16
EVENS = list(range(0, N_QUADS, 2))  # K-columns of the A halves
ODDS = list(range(1, N_QUADS, 2))   # K-columns of the B halves

LAST_RESULTS = None  # BassKernelResults of the most recent run (for test.py)


def setup_tracing():
    """Register the axon NTFF profile hook; the container's antenv is a stub."""
    try:
        import antenv
        if "antenv.axon_hooks" not in sys.modules:
            mod = types.ModuleType("antenv.axon_hooks")
            mod._hook = None
            mod.set_axon_ntff_profile_hook = lambda h: setattr(mod, "_hook", h)
            mod.get_axon_ntff_profile_hook = lambda: mod._hook
            sys.modules["antenv.axon_hooks"] = mod
            antenv.axon_hooks = mod
            from trn_agent_boot.trn_boot import _ntff_profile_via_ctypes
            mod.set_axon_ntff_profile_hook(
                _ntff_profile_via_ctypes("/opt/axon/libaxon_pjrt.so")
            )
        bass_utils.upload_artifacts = lambda tmpdir: f"local://{tmpdir}"
    except Exception:
        pass


def build_nc():
    nc = bacc.Bacc(None, target_bir_lowering=False, num_devices=N_CORES)
    f32 = mybir.dt.float32
    bf16 = mybir.dt.bfloat16
    warm_bursts = int(os.environ.get("WARM_BURSTS", "7"))
    warm_per = int(os.environ.get("WARM_PER", "42"))
    pace_cols = int(os.environ.get("PACE_COLS", "4096"))

    # Per-core inputs (shapes identical on every core; contents sharded).
    wt = nc.dram_tensor("wt", [N_NODES, LOCAL], bf16, kind="ExternalInput")
    xb = nc.dram_tensor("xb", [128, CHUNK_F], f32, kind="ExternalInput")
    s_in = nc.dram_tensor("s_in", [128, BATCH], bf16, kind="ExternalInput")
    out = nc.dram_tensor("out", [128, CHUNK_F], f32, kind="ExternalOutput")

    with tile.TileContext(nc) as tc:
        with (
            tc.tile_pool(name="persist", bufs=1) as persist,
            tc.tile_pool(name="ys", bufs=2) as ys_pool,
            tc.tile_pool(name="chain", bufs=2) as chain,
            tc.tile_pool(name="stage", bufs=2) as stage_pool,
            tc.tile_pool(name="psum", bufs=2, space="PSUM") as psum_pool,
            tc.tile_pool(name="psumt", bufs=2, space="PSUM") as psumt_pool,
            tc.tile_pool(name="dram", bufs=2, space="DRAM") as dram,
        ):
            # ---- persistent SBUF tensors -------------------------------
            xb_sb = persist.tile([128, CHUNK_F], f32)
            nc.sync.dma_start(out=xb_sb, in_=xb[:])
            s_sb = persist.tile([128, BATCH], bf16)
            nc.scalar.dma_start(out=s_sb, in_=s_in[:])
            wt_sb = persist.tile([128, K_TILES, LOCAL], bf16)      # 128 KB/part
            wt_v = wt.rearrange("(t p) n -> p t n", p=128)
            for i in range(8):
                eng = nc.sync if i % 2 == 0 else nc.scalar
                eng.dma_start(
                    out=wt_sb[:, 8 * i : 8 * (i + 1), :],
                    in_=wt_v[:, 8 * i : 8 * (i + 1), :],
                )
            x_buf = persist.tile([128, K_TILES * BATCH], bf16)
            out_f32 = persist.tile([128, CHUNK_F], f32)

            def activation(z_src, to_bf, also_f32=None, width=HALF_F):
                """to_bf[:] = mml(z_src); optionally also f32 copy.

                mml(z) = min(LeakyRelu_leak(z), 1 - 0.25/max(z, 0.5))
                (exact for |z| < ~99, which holds here).  LeakyRelu branch
                on GPSIMD overlaps the DVE saturation-branch chain.
                """
                lr_t = chain.tile([128, width], f32, tag="lr", name="lr_t")
                nc.scalar.activation(
                    lr_t, z_src, mybir.ActivationFunctionType.Lrelu,
                    alpha=LEAK,
                )
                m_t = chain.tile([128, width], f32, tag="m", name="m_t")
                nc.vector.tensor_scalar_max(m_t, z_src, 0.5)
                r_t = chain.tile([128, width], f32, tag="r", name="r_t")
                nc.vector.reciprocal_approx_fast(out=r_t, in_=m_t)
                s_t = chain.tile([128, width], f32, tag="s", name="s_t")
                nc.vector.tensor_scalar(
                    s_t, r_t, -0.25, 1.0,
                    mybir.AluOpType.mult, mybir.AluOpType.add,
                )
                nc.vector.tensor_tensor(to_bf, lr_t, s_t, mybir.AluOpType.min)
                if also_f32 is not None:
                    nc.vector.tensor_tensor(
                        also_f32, lr_t, s_t, mybir.AluOpType.min
                    )

            def tail_half(psum_hv, v, write_out):
                """Reduce+transpose (S-matrix PE pass), bias+activation for
                output half v; returns the staged bf16 (128, HALF_F) tile."""
                psum_t = psumt_pool.tile(
                    [128, HALF_F], mybir.dt.float32, tag="pt", name="psum_t"
                )
                for tt_ in range(4):
                    ysb = ys_pool.tile(
                        [128, 128], bf16, tag=f"ys{tt_}", name=f"ysb{tt_}"
                    )
                    nc.vector.tensor_copy(ysb, psum_hv[:, ts(tt_, 128)])
                    nc.tensor.matmul(
                        psum_t[:, ts(tt_, BATCH)],
                        ysb,
                        s_sb,
                        start=True,
                        stop=True,
                    )
                hs = ts(v, HALF_F)
                z_t = chain.tile([128, HALF_F], mybir.dt.float32,
                                 tag="z", name="z_t")
                nc.vector.tensor_tensor(
                    z_t, psum_t, xb_sb[:, hs], mybir.AluOpType.add
                )
                stage_v = stage_pool.tile(
                    [128, HALF_F], bf16, tag=f"st{v}", name=f"stage{v}"
                )
                activation(
                    z_t,
                    stage_v,
                    also_f32=out_f32[:, hs] if write_out else None,
                )
                return stage_v

            def broadcast_half(stage_v, v):
                """AllGather one staged half into x_buf's half-v columns."""
                ag_in = dram.tile([128, HALF_F], bf16, tag=f"agi{v}",
                                  name=f"ag_in{v}")
                nc.sync.dma_start(out=ag_in, in_=stage_v)
                ag_out = dram.tile(
                    [128 * N_CORES, HALF_F], bf16, addr_space="Shared",
                    tag=f"ago{v}", name=f"ag_out{v}",
                )
                nc.gpsimd.collective_compute(
                    "AllGather",
                    mybir.AluOpType.bypass,
                    replica_groups=[list(range(N_CORES))],
                    ins=[ag_in.opt()],
                    outs=[ag_out.opt()],
                )
                # strided unload: chunk c -> x_buf cols [256c+128v, +128);
                # split 2-way so the first fresh quads' data lands sooner
                dst_v = x_buf.rearrange("p (c f) -> p c f", c=N_CORES)[
                    :, :, HALF_F * v : HALF_F * (v + 1)
                ]
                src_v = ag_out.rearrange("(c p) f -> p c f", p=128)
                nc.sync.dma_start(out=dst_v[:, 0:4], in_=src_v[:, 0:4])
                nc.scalar.dma_start(out=dst_v[:, 4:], in_=src_v[:, 4:])

            def pe_warm():
                """Unpaced dummy matmuls filling most of the collective
                window so HAM never sees a >3.4us PE idle gap (which would
                halve the PE clock).  Sized to end ~3us before the fresh
                quads' data lands: the residual idle stays under the HAM
                threshold and an early collective is never blocked."""
                if warm_per <= 0:
                    return
                psum_w = psumt_pool.tile(
                    [128, 512], mybir.dt.float32, tag="pw", name="psum_w",
                    bufs=1,
                )
                for _ in range(warm_per):
                    nc.tensor.matmul(
                        psum_w[0:BATCH, :], s_sb, wt_sb[:, 0, 0:512],
                        start=True, stop=True, skip_group_check=True,
                    )

            def mm_quads(h, psum_hv, quads, start, stop):
                for qi, q in enumerate(quads):
                    for j in range(4):
                        k = 4 * q + j
                        nc.tensor.matmul(
                            psum_hv[32 * j : 32 * (j + 1), :],
                            x_buf[:, ts(k, BATCH)],
                            wt_sb[:, k, ts(h, 512)],
                            start=start and qi == 0,
                            stop=stop and qi == len(quads) - 1,
                            tile_position=(0, 32 * j),
                        )

            # ---- Gauss-Seidel half-steps -------------------------------
            # t odd: update half A (h=0);  t even: update half B (h=1).
            # Half-step t uses the fresh other half (gathered at t-1) and
            # its own 2-old half (gathered at t-2).
            for t in range(1, GS_HALF_STEPS + 1):
                h = (t + 1) % 2
                last = t == GS_HALF_STEPS
                write_out = t >= GS_HALF_STEPS - 1
                if t == 1:
                    # A(1) = mml(xb_A): state is zero, no matmuls
                    stage_v = stage_pool.tile(
                        [128, HALF_F], bf16, tag="st0", name="stage0"
                    )
                    activation(xb_sb[:, ts(0, HALF_F)], stage_v,
                               also_f32=out_f32[:, ts(0, HALF_F)]
                               if write_out else None)
                else:
                    stale = EVENS if h == 0 else ODDS
                    fresh = ODDS if h == 0 else EVENS
                    psum_hv = psum_pool.tile(
                        [128, 512], mybir.dt.float32, tag="pm", name="psum_m"
                    )
                    if t == 2:
                        # B(2) = mml(W_BA A(1) + xb_B): only A-columns
                        pe_warm()
                        mm_quads(1, psum_hv, EVENS, start=True, stop=True)
                    else:
                        mm_quads(h, psum_hv, stale, start=True, stop=False)
                        # paced warm fills the PE gap while the fresh
                        # half's collective is still in flight
                        pe_warm()
                        mm_quads(h, psum_hv, fresh, start=False, stop=True)
                    stage_v = tail_half(psum_hv, h, write_out)
                if last:
                    nc.sync.dma_start(out=out[:], in_=out_f32)
                else:
                    broadcast_half(stage_v, h)

    nc.compile()
    return nc


def _prepare_in_maps(X_full, weights, bias, edge_mask):
    W = np.where(edge_mask, weights, 0.0).astype(np.float32)
    Xb = X_full.astype(np.float32).T + bias.astype(np.float32)  # (n, B)
    S = np.zeros((128, BATCH), np.float32)
    S[np.arange(128), np.arange(128) % BATCH] = 1.0
    S = S.astype(ml_dtypes.bfloat16)
    in_maps = []
    for c in range(N_CORES):
        rows = slice(LOCAL * c, LOCAL * (c + 1))
        wt_c = np.ascontiguousarray(W[rows, :].T).astype(ml_dtypes.bfloat16)
        xb_c = (
            Xb[rows]                       # (1024, 32)
            .reshape(LOCAL_TILES, 128, BATCH)
            .transpose(1, 0, 2)
            .reshape(128, CHUNK_F)
            .copy()
        )
        in_maps.append({"wt": wt_c, "xb": xb_c, "s_in": S})
    return in_maps


def _reassemble(results):
    out = np.empty((BATCH, N_NODES), np.float32)
    for c in range(N_CORES):
        oc = np.asarray(results[c]["out"])  # (128, 256)
        chunk = (
            oc.reshape(128, LOCAL_TILES, BATCH)
            .transpose(1, 0, 2)
            .reshape(LOCAL, BATCH)
        )
        out[:, LOCAL * c : LOCAL * (c + 1)] = chunk.T
    return out


def kernel(X_full, weights, bias, edge_mask):
    global LAST_RESULTS
    setup_tracing()
    in_maps = _prepare_in_maps(X_full, weights, bias, edge_mask)
    nc = build_nc()
    res = run_bass_kernel_spmd(nc, in_maps, core_ids=list(range(N_CORES)))
    LAST_RESULTS = res
    return _reassemble(res.results)


if __name__ == "__main__":
    # quick self-run with random data
    rng = np.random.default_rng(0)
    X_full = rng.random((BATCH, N_NODES), np.float32)
    weights = rng.standard_normal((N_NODES, N_NODES), np.float32)
    bias = 0.001 * np.ones((N_NODES, 1), np.float32)
    edge_mask = rng.random((N_NODES, N_NODES)) < 0.002
    out = kernel(X_full, weights, bias, edge_mask)
    print("out", out.shape, out.dtype, out[:2, :4])


# revision 13
# speedup vs baseline: 1.2173x; 1.0993x over previous
"""Trainium2 Bass kernel for BioNet message-passing recurrence.

Reference computes 50 steps of Jacobi iteration  X <- mml(W @ X + X_bias)
with W (8192x8192 f32, masked) and X (8192x32), returning X.T (32, 8192).
The iteration is a contraction converging to a fixed point; ANY update
schedule converging to the same fixed point gives the same answer.  The
kernel uses block GAUSS-SEIDEL over two global half-blocks (nodes
[1024c,1024c+512) = half A, rest = half B): updating one half per
"half-step" using the freshest available other half converges in 6
sweeps (12 half-steps, verified to the bf16 noise floor) vs 11+ Jacobi
steps, and needs only ONE AllGather per half-step — which matters
because each collective costs ~6-8us of mostly-fixed protocol latency
on the single CC core.

Per-core layout (8 NeuronCores, tensor-parallel over W rows):
  - A tiny warmup AllGather is dispatched first so the one-time NRT
    comm-init / core start-skew cost (~70us) overlaps the W load.
  - Each core holds rows [1024c, 1024c+1024) of W transposed in SBUF as
    bf16 (16.8 MB/core); DMAed in 8 K-chunks so step 2 starts early.
  - Half-step updating half U: 16 quads of out^T = X^T @ W^T with X
    (128,32) tiles stationary, 4-way column-tiled (tile_position) for
    ~4x PE throughput at batch=32.  Quads over U-parity K-columns use
    the 2-half-steps-old U state (long available, pre-run during the
    previous collective's flight); quads over the other parity wait for
    the just-gathered fresh half.
  - The 4 column-group partials land on partition groups 32j..32j+32 of
    PSUM; a small PE pass with selector S[p,b] = (p%32==b) fuses the
    4-way reduction with the (batch,node)->(node,batch) transpose; the
    PSUM->SBUF copy is split 4-ways to pipeline with the S-mms.
  - Activation uses  mml(z) = min(LeakyRelu_leak(z), 1-0.25/max(z,0.5))
    (algebraically exact); the LeakyRelu branch runs on GPSIMD in
    parallel with the saturation branch on DVE.
  - Queues: staging SBUF->DRAM and unload DMAs on sync (the unload's
    collective wait parks after the staging it follows, blocking
    nothing), collective triggers on gpsimd, LeakyRelu on the scalar
    queue, so no critical op ever queues behind a collective wait.
  - Paced dummy-matmul bursts keep the PE from idling >3.4us during the
    collective window, which would trip HAM clock-throttling (halves
    the PE clock).
"""

import os
import sys
import types

sys.path.insert(0, "/opt/trn_rl_repo")

import numpy as np
import ml_dtypes

import concourse.bass as bass
import concourse.mybir as mybir
import concourse.tile as tile
from concourse import bacc
import concourse.bass_utils as bass_utils
from concourse.bass import ts
from concourse.bass_utils import run_bass_kernel_spmd

N_NODES = 8192
N_CORES = 8
BATCH = 32
GS_HALF_STEPS = 13                  # 6.5 Gauss-Seidel sweeps (see module doc)
LEAK = 0.01
LOCAL = N_NODES // N_CORES          # 1024 rows per core
K_TILES = N_NODES // 128            # 64
LOCAL_TILES = LOCAL // 128          # 8
CHUNK_F = LOCAL_TILES * BATCH       # 256 free elems per activated chunk
HALF_F = CHUNK_F // 2               # 128
N_QUADS = K_TILES // 4              # 16
EVENS = list(range(0, N_QUADS, 2))  # K-columns of the A halves
ODDS = list(range(1, N_QUADS, 2))   # K-columns of the B halves

LAST_RESULTS = None  # BassKernelResults of the most recent run (for test.py)


def setup_tracing():
    """Register the axon NTFF profile hook; the container's antenv is a stub."""
    try:
        import antenv
        if "antenv.axon_hooks" not in sys.modules:
            mod = types.ModuleType("antenv.axon_hooks")
            mod._hook = None
            mod.set_axon_ntff_profile_hook = lambda h: setattr(mod, "_hook", h)
            mod.get_axon_ntff_profile_hook = lambda: mod._hook
            sys.modules["antenv.axon_hooks"] = mod
            antenv.axon_hooks = mod
            from trn_agent_boot.trn_boot import _ntff_profile_via_ctypes
            mod.set_axon_ntff_profile_hook(
                _ntff_profile_via_ctypes("/opt/axon/libaxon_pjrt.so")
            )
        bass_utils.upload_artifacts = lambda tmpdir: f"local://{tmpdir}"
    except Exception:
        pass


def build_nc():
    nc = bacc.Bacc(None, target_bir_lowering=False, num_devices=N_CORES)
    f32 = mybir.dt.float32
    bf16 = mybir.dt.bfloat16
    warm_bursts = int(os.environ.get("WARM_BURSTS", "7"))
    warm_per = int(os.environ.get("WARM_PER", "30"))
    pace_cols = int(os.environ.get("PACE_COLS", "4096"))

    # Per-core inputs (shapes identical on every core; contents sharded).
    wt = nc.dram_tensor("wt", [N_NODES, LOCAL], bf16, kind="ExternalInput")
    xb = nc.dram_tensor("xb", [128, CHUNK_F], f32, kind="ExternalInput")
    s_in = nc.dram_tensor("s_in", [128, BATCH], bf16, kind="ExternalInput")
    out = nc.dram_tensor("out", [128, CHUNK_F], f32, kind="ExternalOutput")

    with tile.TileContext(nc) as tc:
        with (
            tc.tile_pool(name="persist", bufs=1) as persist,
            tc.tile_pool(name="ys", bufs=2) as ys_pool,
            tc.tile_pool(name="chain", bufs=2) as chain,
            tc.tile_pool(name="stage", bufs=2) as stage_pool,
            tc.tile_pool(name="psum", bufs=2, space="PSUM") as psum_pool,
            tc.tile_pool(name="psumt", bufs=2, space="PSUM") as psumt_pool,
            tc.tile_pool(name="dram", bufs=2, space="DRAM") as dram,
        ):
            # ---- persistent SBUF tensors -------------------------------
            xb_sb = persist.tile([128, CHUNK_F], f32)
            nc.sync.dma_start(out=xb_sb, in_=xb[:])
            s_sb = persist.tile([128, BATCH], bf16)
            nc.scalar.dma_start(out=s_sb, in_=s_in[:])
            wt_sb = persist.tile([128, K_TILES, LOCAL], bf16)      # 128 KB/part
            wt_v = wt.rearrange("(t p) n -> p t n", p=128)
            for i in range(8):
                eng = nc.sync if i % 2 == 0 else nc.scalar
                eng.dma_start(
                    out=wt_sb[:, 8 * i : 8 * (i + 1), :],
                    in_=wt_v[:, 8 * i : 8 * (i + 1), :],
                )
            x_buf = persist.tile([128, K_TILES * BATCH], bf16)
            out_f32 = persist.tile([128, CHUNK_F], f32)

            def activation(z_src, to_bf, also_f32=None, width=HALF_F):
                """to_bf[:] = mml(z_src); optionally also f32 copy.

                mml(z) = min(LeakyRelu_leak(z), 1 - 0.25/max(z, 0.5))
                (exact for |z| < ~99, which holds here).  LeakyRelu branch
                on GPSIMD overlaps the DVE saturation-branch chain.
                """
                lr_t = chain.tile([128, width], f32, tag="lr", name="lr_t")
                nc.scalar.activation(
                    lr_t, z_src, mybir.ActivationFunctionType.Lrelu,
                    alpha=LEAK,
                )
                m_t = chain.tile([128, width], f32, tag="m", name="m_t")
                nc.vector.tensor_scalar_max(m_t, z_src, 0.5)
                r_t = chain.tile([128, width], f32, tag="r", name="r_t")
                nc.vector.reciprocal_approx_fast(out=r_t, in_=m_t)
                s_t = chain.tile([128, width], f32, tag="s", name="s_t")
                nc.vector.tensor_scalar(
                    s_t, r_t, -0.25, 1.0,
                    mybir.AluOpType.mult, mybir.AluOpType.add,
                )
                nc.vector.tensor_tensor(to_bf, lr_t, s_t, mybir.AluOpType.min)
                if also_f32 is not None:
                    nc.vector.tensor_tensor(
                        also_f32, lr_t, s_t, mybir.AluOpType.min
                    )

            def tail_half(psum_hv, v, write_out):
                """Reduce+transpose (S-matrix PE pass), bias+activation for
                output half v; returns the staged bf16 (128, HALF_F) tile."""
                psum_t = psumt_pool.tile(
                    [128, HALF_F], mybir.dt.float32, tag="pt", name="psum_t"
                )
                for tt_ in range(4):
                    ysb = ys_pool.tile(
                        [128, 128], bf16, tag=f"ys{tt_}", name=f"ysb{tt_}"
                    )
                    nc.vector.tensor_copy(ysb, psum_hv[:, ts(tt_, 128)])
                    nc.tensor.matmul(
                        psum_t[:, ts(tt_, BATCH)],
                        ysb,
                        s_sb,
                        start=True,
                        stop=True,
                    )
                hs = ts(v, HALF_F)
                z_t = chain.tile([128, HALF_F], mybir.dt.float32,
                                 tag="z", name="z_t")
                nc.vector.tensor_tensor(
                    z_t, psum_t, xb_sb[:, hs], mybir.AluOpType.add
                )
                stage_v = stage_pool.tile(
                    [128, HALF_F], bf16, tag=f"st{v}", name=f"stage{v}"
                )
                activation(
                    z_t,
                    stage_v,
                    also_f32=out_f32[:, hs] if write_out else None,
                )
                return stage_v

            def broadcast_half(stage_v, v):
                """AllGather one staged half into x_buf's half-v columns."""
                ag_in = dram.tile([128, HALF_F], bf16, tag=f"agi{v}",
                                  name=f"ag_in{v}")
                nc.sync.dma_start(out=ag_in, in_=stage_v)
                ag_out = dram.tile(
                    [128 * N_CORES, HALF_F], bf16, addr_space="Shared",
                    tag=f"ago{v}", name=f"ag_out{v}",
                )
                nc.gpsimd.collective_compute(
                    "AllGather",
                    mybir.AluOpType.bypass,
                    replica_groups=[list(range(N_CORES))],
                    ins=[ag_in.opt()],
                    outs=[ag_out.opt()],
                )
                # strided unload: chunk c -> x_buf cols [256c+128v, +128);
                # split 2-way so the first fresh quads' data lands sooner
                dst_v = x_buf.rearrange("p (c f) -> p c f", c=N_CORES)[
                    :, :, HALF_F * v : HALF_F * (v + 1)
                ]
                src_v = ag_out.rearrange("(c p) f -> p c f", p=128)
                nc.sync.dma_start(out=dst_v[:, 0:4], in_=src_v[:, 0:4])
                nc.scalar.dma_start(out=dst_v[:, 4:], in_=src_v[:, 4:])

            def pe_warm():
                """Unpaced dummy matmuls filling most of the collective
                window so HAM never sees a >3.4us PE idle gap (which would
                halve the PE clock).  Sized to end ~3us before the fresh
                quads' data lands: the residual idle stays under the HAM
                threshold and an early collective is never blocked."""
                if warm_per <= 0:
                    return
                psum_w = psumt_pool.tile(
                    [128, 512], mybir.dt.float32, tag="pw", name="psum_w",
                    bufs=1,
                )
                # same 4-way tile_position mode as the mains: full-grid
                # matmuls here would break the quad-concurrency streaks
                # of neighbouring main matmuls (PE mode switch)
                for i in range(warm_per):
                    for j in range(4):
                        nc.tensor.matmul(
                            psum_w[32 * j : 32 * (j + 1), :],
                            s_sb,
                            wt_sb[:, j, 0:512],
                            start=True, stop=True, skip_group_check=True,
                            tile_position=(0, 32 * j),
                        )

            def mm_quads(h, psum_hv, quads, start, stop):
                for qi, q in enumerate(quads):
                    for j in range(4):
                        k = 4 * q + j
                        nc.tensor.matmul(
                            psum_hv[32 * j : 32 * (j + 1), :],
                            x_buf[:, ts(k, BATCH)],
                            wt_sb[:, k, ts(h, 512)],
                            start=start and qi == 0,
                            stop=stop and qi == len(quads) - 1,
                            tile_position=(0, 32 * j),
                        )

            # ---- Gauss-Seidel half-steps -------------------------------
            # t odd: update half A (h=0);  t even: update half B (h=1).
            # Half-step t uses the fresh other half (gathered at t-1) and
            # its own 2-old half (gathered at t-2).
            for t in range(1, GS_HALF_STEPS + 1):
                h = (t + 1) % 2
                last = t == GS_HALF_STEPS
                write_out = t >= GS_HALF_STEPS - 1
                if t == 1:
                    # A(1) = mml(xb_A): state is zero, no matmuls
                    stage_v = stage_pool.tile(
                        [128, HALF_F], bf16, tag="st0", name="stage0"
                    )
                    activation(xb_sb[:, ts(0, HALF_F)], stage_v,
                               also_f32=out_f32[:, ts(0, HALF_F)]
                               if write_out else None)
                else:
                    stale = EVENS if h == 0 else ODDS
                    fresh = ODDS if h == 0 else EVENS
                    psum_hv = psum_pool.tile(
                        [128, 512], mybir.dt.float32, tag="pm", name="psum_m"
                    )
                    if t == 2:
                        # B(2) = mml(W_BA A(1) + xb_B): only A-columns
                        pe_warm()
                        mm_quads(1, psum_hv, EVENS, start=True, stop=True)
                    else:
                        mm_quads(h, psum_hv, stale, start=True, stop=False)
                        # paced warm fills the PE gap while the fresh
                        # half's collective is still in flight
                        pe_warm()
                        mm_quads(h, psum_hv, fresh, start=False, stop=True)
                    stage_v = tail_half(psum_hv, h, write_out)
                if last:
                    nc.sync.dma_start(out=out[:], in_=out_f32)
                else:
                    broadcast_half(stage_v, h)

    nc.compile()
    return nc


def _prepare_in_maps(X_full, weights, bias, edge_mask):
    W = np.where(edge_mask, weights, 0.0).astype(np.float32)
    Xb = X_full.astype(np.float32).T + bias.astype(np.float32)  # (n, B)
    S = np.zeros((128, BATCH), np.float32)
    S[np.arange(128), np.arange(128) % BATCH] = 1.0
    S = S.astype(ml_dtypes.bfloat16)
    in_maps = []
    for c in range(N_CORES):
        rows = slice(LOCAL * c, LOCAL * (c + 1))
        wt_c = np.ascontiguousarray(W[rows, :].T).astype(ml_dtypes.bfloat16)
        xb_c = (
            Xb[rows]                       # (1024, 32)
            .reshape(LOCAL_TILES, 128, BATCH)
            .transpose(1, 0, 2)
            .reshape(128, CHUNK_F)
            .copy()
        )
        in_maps.append({"wt": wt_c, "xb": xb_c, "s_in": S})
    return in_maps


def _reassemble(results):
    out = np.empty((BATCH, N_NODES), np.float32)
    for c in range(N_CORES):
        oc = np.asarray(results[c]["out"])  # (128, 256)
        chunk = (
            oc.reshape(128, LOCAL_TILES, BATCH)
            .transpose(1, 0, 2)
            .reshape(LOCAL, BATCH)
        )
        out[:, LOCAL * c : LOCAL * (c + 1)] = chunk.T
    return out


def kernel(X_full, weights, bias, edge_mask):
    global LAST_RESULTS
    setup_tracing()
    in_maps = _prepare_in_maps(X_full, weights, bias, edge_mask)
    nc = build_nc()
    res = run_bass_kernel_spmd(nc, in_maps, core_ids=list(range(N_CORES)))
    LAST_RESULTS = res
    return _reassemble(res.results)


if __name__ == "__main__":
    # quick self-run with random data
    rng = np.random.default_rng(0)
    X_full = rng.random((BATCH, N_NODES), np.float32)
    weights = rng.standard_normal((N_NODES, N_NODES), np.float32)
    bias = 0.001 * np.ones((N_NODES, 1), np.float32)
    edge_mask = rng.random((N_NODES, N_NODES)) < 0.002
    out = kernel(X_full, weights, bias, edge_mask)
    print("out", out.shape, out.dtype, out[:2, :4])


# revision 17
# speedup vs baseline: 1.3269x; 1.0901x over previous
"""Trainium2 Bass kernel for BioNet message-passing recurrence.

Reference computes 50 steps of Jacobi iteration  X <- mml(W @ X + X_bias)
with W (8192x8192 f32, masked) and X (8192x32), returning X.T (32, 8192).
The iteration is a contraction converging to a fixed point; ANY update
schedule converging to the same fixed point gives the same answer.  The
kernel uses block GAUSS-SEIDEL over two global half-blocks (nodes
[1024c,1024c+512) = half A, rest = half B): updating one half per
"half-step" using the freshest available other half converges in 6
sweeps (12 half-steps, verified to the bf16 noise floor) vs 11+ Jacobi
steps, and needs only ONE AllGather per half-step — which matters
because each collective costs ~6-8us of mostly-fixed protocol latency
on the single CC core.

Per-core layout (8 NeuronCores, tensor-parallel over W rows):
  - A tiny warmup AllGather is dispatched first so the one-time NRT
    comm-init / core start-skew cost (~70us) overlaps the W load.
  - Each core holds rows [1024c, 1024c+1024) of W transposed in SBUF as
    bf16 (16.8 MB/core); DMAed in 8 K-chunks so step 2 starts early.
  - Half-step updating half U: 16 quads of out^T = X^T @ W^T with X
    (128,32) tiles stationary, 4-way column-tiled (tile_position) for
    ~4x PE throughput at batch=32.  Quads over U-parity K-columns use
    the 2-half-steps-old U state (long available, pre-run during the
    previous collective's flight); quads over the other parity wait for
    the just-gathered fresh half.
  - The 4 column-group partials land on partition groups 32j..32j+32 of
    PSUM; a small PE pass with selector S[p,b] = (p%32==b) fuses the
    4-way reduction with the (batch,node)->(node,batch) transpose; the
    PSUM->SBUF copy is split 4-ways to pipeline with the S-mms.
  - Activation uses  mml(z) = min(LeakyRelu_leak(z), 1-0.25/max(z,0.5))
    (algebraically exact); the LeakyRelu branch runs on GPSIMD in
    parallel with the saturation branch on DVE.
  - Queues: staging SBUF->DRAM and unload DMAs on sync (the unload's
    collective wait parks after the staging it follows, blocking
    nothing), collective triggers on gpsimd, LeakyRelu on the scalar
    queue, so no critical op ever queues behind a collective wait.
  - Paced dummy-matmul bursts keep the PE from idling >3.4us during the
    collective window, which would trip HAM clock-throttling (halves
    the PE clock).
"""

import os
import sys
import types

sys.path.insert(0, "/opt/trn_rl_repo")

import numpy as np
import ml_dtypes

import concourse.bass as bass
import concourse.mybir as mybir
import concourse.tile as tile
from concourse import bacc
import concourse.bass_utils as bass_utils
from concourse.bass import ts
from concourse.bass_utils import run_bass_kernel_spmd

N_NODES = 8192
N_CORES = 8
BATCH = 32
GS_HALF_STEPS = 12                  # 6 Gauss-Seidel sweeps (see module doc)
LEAK = 0.01
LOCAL = N_NODES // N_CORES          # 1024 rows per core
K_TILES = N_NODES // 128            # 64
LOCAL_TILES = LOCAL // 128          # 8
CHUNK_F = LOCAL_TILES * BATCH       # 256 free elems per activated chunk
HALF_F = CHUNK_F // 2               # 128
N_QUADS = K_TILES // 4              # 16
EVENS = list(range(0, N_QUADS, 2))  # K-columns of the A halves
ODDS = list(range(1, N_QUADS, 2))   # K-columns of the B halves
# SBUF W image K-tile order: even-quad tiles (A-half sources) first
K_ORDER = [4 * q + j for q in EVENS + ODDS for j in range(4)]
K_SLOT = {k: i for i, k in enumerate(K_ORDER)}

LAST_RESULTS = None  # BassKernelResults of the most recent run (for test.py)


def setup_tracing():
    """Register the axon NTFF profile hook; the container's antenv is a stub."""
    try:
        import antenv
        if "antenv.axon_hooks" not in sys.modules:
            mod = types.ModuleType("antenv.axon_hooks")
            mod._hook = None
            mod.set_axon_ntff_profile_hook = lambda h: setattr(mod, "_hook", h)
            mod.get_axon_ntff_profile_hook = lambda: mod._hook
            sys.modules["antenv.axon_hooks"] = mod
            antenv.axon_hooks = mod
            from trn_agent_boot.trn_boot import _ntff_profile_via_ctypes
            mod.set_axon_ntff_profile_hook(
                _ntff_profile_via_ctypes("/opt/axon/libaxon_pjrt.so")
            )
        bass_utils.upload_artifacts = lambda tmpdir: f"local://{tmpdir}"
    except Exception:
        pass


def build_nc():
    nc = bacc.Bacc(None, target_bir_lowering=False, num_devices=N_CORES)
    f32 = mybir.dt.float32
    bf16 = mybir.dt.bfloat16
    warm_bursts = int(os.environ.get("WARM_BURSTS", "7"))
    warm_per = int(os.environ.get("WARM_PER", "40"))
    pace_cols = int(os.environ.get("PACE_COLS", "4096"))

    # Per-core inputs (shapes identical on every core; contents sharded).
    wt = nc.dram_tensor("wt", [N_NODES, LOCAL], bf16, kind="ExternalInput")
    xb = nc.dram_tensor("xb", [128, CHUNK_F], f32, kind="ExternalInput")
    s_in = nc.dram_tensor("s_in", [128, BATCH], bf16, kind="ExternalInput")
    out = nc.dram_tensor("out", [128, CHUNK_F], f32, kind="ExternalOutput")

    with tile.TileContext(nc) as tc:
        with (
            tc.tile_pool(name="persist", bufs=1) as persist,
            tc.tile_pool(name="ys", bufs=2) as ys_pool,
            tc.tile_pool(name="chain", bufs=2) as chain,
            tc.tile_pool(name="stage", bufs=2) as stage_pool,
            tc.tile_pool(name="psum", bufs=2, space="PSUM") as psum_pool,
            tc.tile_pool(name="psumt", bufs=2, space="PSUM") as psumt_pool,
            tc.tile_pool(name="dram", bufs=2, space="DRAM") as dram,
        ):
            # ---- persistent SBUF tensors -------------------------------
            xb_sb = persist.tile([128, CHUNK_F], f32)
            nc.sync.dma_start(out=xb_sb, in_=xb[:])
            s_sb = persist.tile([128, BATCH], bf16)
            nc.scalar.dma_start(out=s_sb, in_=s_in[:])
            wt_sb = persist.tile([128, K_TILES, LOCAL], bf16)      # 128 KB/part
            # host stores K-tiles with even-quad (A-half source) tiles in
            # slots 0..31 and odd-quad tiles in 32..63 (K_SLOT): half-step
            # 2 only needs the first half of the image, so it starts after
            # ~half the 16.8 MB load
            wt_v = wt.rearrange("(t p) n -> p t n", p=128)
            for i in range(8):
                eng = nc.sync if i % 2 == 0 else nc.scalar
                eng.dma_start(
                    out=wt_sb[:, 8 * i : 8 * (i + 1), :],
                    in_=wt_v[:, 8 * i : 8 * (i + 1), :],
                )
            x_buf = persist.tile([128, K_TILES * BATCH], bf16)
            out_f32 = persist.tile([128, CHUNK_F], f32)

            def activation(z_src, to_bf, also_f32=None, width=HALF_F):
                """to_bf[:] = mml(z_src); optionally also f32 copy.

                mml(z) = min(LeakyRelu_leak(z), 1 - 0.25/max(z, 0.5))
                (exact for |z| < ~99, which holds here).  LeakyRelu branch
                on GPSIMD overlaps the DVE saturation-branch chain.
                """
                lr_t = chain.tile([128, width], f32, tag="lr", name="lr_t")
                nc.scalar.activation(
                    lr_t, z_src, mybir.ActivationFunctionType.Lrelu,
                    alpha=LEAK,
                )
                m_t = chain.tile([128, width], f32, tag="m", name="m_t")
                nc.vector.tensor_scalar_max(m_t, z_src, 0.5)
                r_t = chain.tile([128, width], f32, tag="r", name="r_t")
                nc.vector.reciprocal_approx_fast(out=r_t, in_=m_t)
                s_t = chain.tile([128, width], f32, tag="s", name="s_t")
                nc.vector.tensor_scalar(
                    s_t, r_t, -0.25, 1.0,
                    mybir.AluOpType.mult, mybir.AluOpType.add,
                )
                nc.vector.tensor_tensor(to_bf, lr_t, s_t, mybir.AluOpType.min)
                if also_f32 is not None:
                    nc.vector.tensor_tensor(
                        also_f32, lr_t, s_t, mybir.AluOpType.min
                    )

            def tail_half(psum_hv, v, write_out):
                """Reduce+transpose (S-matrix PE pass), bias+activation for
                output half v; returns the staged bf16 (128, HALF_F) tile."""
                psum_t = psumt_pool.tile(
                    [128, HALF_F], mybir.dt.float32, tag="pt", name="psum_t"
                )
                for tt_ in range(4):
                    ysb = ys_pool.tile(
                        [128, 128], bf16, tag=f"ys{tt_}", name=f"ysb{tt_}"
                    )
                    nc.vector.tensor_copy(ysb, psum_hv[:, ts(tt_, 128)])
                    nc.tensor.matmul(
                        psum_t[:, ts(tt_, BATCH)],
                        ysb,
                        s_sb,
                        start=True,
                        stop=True,
                    )
                hs = ts(v, HALF_F)
                z_t = chain.tile([128, HALF_F], mybir.dt.float32,
                                 tag="z", name="z_t")
                nc.vector.tensor_tensor(
                    z_t, psum_t, xb_sb[:, hs], mybir.AluOpType.add
                )
                stage_v = stage_pool.tile(
                    [128, HALF_F], bf16, tag=f"st{v}", name=f"stage{v}"
                )
                activation(
                    z_t,
                    stage_v,
                    also_f32=out_f32[:, hs] if write_out else None,
                )
                return stage_v

            def broadcast_half(stage_v, v):
                """AllGather one staged half into x_buf's half-v columns."""
                ag_in = dram.tile([128, HALF_F], bf16, tag=f"agi{v}",
                                  name=f"ag_in{v}")
                nc.sync.dma_start(out=ag_in, in_=stage_v)
                ag_out = dram.tile(
                    [128 * N_CORES, HALF_F], bf16, addr_space="Shared",
                    tag=f"ago{v}", name=f"ag_out{v}",
                )
                nc.gpsimd.collective_compute(
                    "AllGather",
                    mybir.AluOpType.bypass,
                    replica_groups=[list(range(N_CORES))],
                    ins=[ag_in.opt()],
                    outs=[ag_out.opt()],
                )
                # strided unload: chunk c -> x_buf cols [256c+128v, +128);
                # split 2-way so the first fresh quads' data lands sooner
                dst_v = x_buf.rearrange("p (c f) -> p c f", c=N_CORES)[
                    :, :, HALF_F * v : HALF_F * (v + 1)
                ]
                src_v = ag_out.rearrange("(c p) f -> p c f", p=128)
                nc.sync.dma_start(out=dst_v[:, 0:3], in_=src_v[:, 0:3])
                nc.scalar.dma_start(out=dst_v[:, 3:6], in_=src_v[:, 3:6])
                nc.gpsimd.dma_start(out=dst_v[:, 6:8], in_=src_v[:, 6:8])

            def pe_warm():
                """Unpaced dummy matmuls filling most of the collective
                window so HAM never sees a >3.4us PE idle gap (which would
                halve the PE clock).  Sized to end ~3us before the fresh
                quads' data lands: the residual idle stays under the HAM
                threshold and an early collective is never blocked."""
                if warm_per <= 0:
                    return
                psum_w = psumt_pool.tile(
                    [128, 512], mybir.dt.float32, tag="pw", name="psum_w",
                    bufs=1,
                )
                # same 4-way tile_position mode as the mains: full-grid
                # matmuls here would break the quad-concurrency streaks
                # of neighbouring main matmuls (PE mode switch)
                for i in range(warm_per):
                    for j in range(4):
                        nc.tensor.matmul(
                            psum_w[32 * j : 32 * (j + 1), :],
                            s_sb,
                            wt_sb[:, j, 0:512],
                            start=True, stop=True, skip_group_check=True,
                            tile_position=(0, 32 * j),
                        )

            def mm_quads(h, psum_hv, quads, start, stop):
                for qi, q in enumerate(quads):
                    for j in range(4):
                        k = 4 * q + j
                        nc.tensor.matmul(
                            psum_hv[32 * j : 32 * (j + 1), :],
                            x_buf[:, ts(k, BATCH)],
                            wt_sb[:, K_SLOT[k], ts(h, 512)],
                            start=start and qi == 0,
                            stop=stop and qi == len(quads) - 1,
                            tile_position=(0, 32 * j),
                        )

            # ---- Gauss-Seidel half-steps -------------------------------
            # t odd: update half A (h=0);  t even: update half B (h=1).
            # Half-step t uses the fresh other half (gathered at t-1) and
            # its own 2-old half (gathered at t-2).
            for t in range(1, GS_HALF_STEPS + 1):
                h = (t + 1) % 2
                last = t == GS_HALF_STEPS
                write_out = t >= GS_HALF_STEPS - 1
                if t == 1:
                    # A(1) = mml(xb_A): state is zero, no matmuls
                    stage_v = stage_pool.tile(
                        [128, HALF_F], bf16, tag="st0", name="stage0"
                    )
                    activation(xb_sb[:, ts(0, HALF_F)], stage_v,
                               also_f32=out_f32[:, ts(0, HALF_F)]
                               if write_out else None)
                else:
                    stale = EVENS if h == 0 else ODDS
                    fresh = ODDS if h == 0 else EVENS
                    psum_hv = psum_pool.tile(
                        [128, 512], mybir.dt.float32, tag="pm", name="psum_m"
                    )
                    if t == 2:
                        # B(2) = mml(W_BA A(1) + xb_B): only A-columns
                        pe_warm()
                        mm_quads(1, psum_hv, EVENS, start=True, stop=True)
                    else:
                        mm_quads(h, psum_hv, stale, start=True, stop=False)
                        # paced warm fills the PE gap while the fresh
                        # half's collective is still in flight
                        pe_warm()
                        mm_quads(h, psum_hv, fresh, start=False, stop=True)
                    stage_v = tail_half(psum_hv, h, write_out)
                if last:
                    nc.sync.dma_start(out=out[:], in_=out_f32)
                else:
                    broadcast_half(stage_v, h)

    nc.compile()
    return nc


def _prepare_in_maps(X_full, weights, bias, edge_mask):
    W = np.where(edge_mask, weights, 0.0).astype(np.float32)
    Xb = X_full.astype(np.float32).T + bias.astype(np.float32)  # (n, B)
    S = np.zeros((128, BATCH), np.float32)
    S[np.arange(128), np.arange(128) % BATCH] = 1.0
    S = S.astype(ml_dtypes.bfloat16)
    in_maps = []
    for c in range(N_CORES):
        rows = slice(LOCAL * c, LOCAL * (c + 1))
        wt_c = np.ascontiguousarray(W[rows, :].T).astype(ml_dtypes.bfloat16)
        # group K-tiles so even-quad (A-half source) tiles come first,
        # matching K_SLOT in the kernel
        wt_c = (
            wt_c.reshape(K_TILES, 128, LOCAL)[K_ORDER]
            .reshape(N_NODES, LOCAL)
        )
        xb_c = (
            Xb[rows]                       # (1024, 32)
            .reshape(LOCAL_TILES, 128, BATCH)
            .transpose(1, 0, 2)
            .reshape(128, CHUNK_F)
            .copy()
        )
        in_maps.append({"wt": wt_c, "xb": xb_c, "s_in": S})
    return in_maps


def _reassemble(results):
    out = np.empty((BATCH, N_NODES), np.float32)
    for c in range(N_CORES):
        oc = np.asarray(results[c]["out"])  # (128, 256)
        chunk = (
            oc.reshape(128, LOCAL_TILES, BATCH)
            .transpose(1, 0, 2)
            .reshape(LOCAL, BATCH)
        )
        out[:, LOCAL * c : LOCAL * (c + 1)] = chunk.T
    return out


def kernel(X_full, weights, bias, edge_mask):
    global LAST_RESULTS
    setup_tracing()
    in_maps = _prepare_in_maps(X_full, weights, bias, edge_mask)
    nc = build_nc()
    res = run_bass_kernel_spmd(nc, in_maps, core_ids=list(range(N_CORES)))
    LAST_RESULTS = res
    return _reassemble(res.results)


if __name__ == "__main__":
    # quick self-run with random data
    rng = np.random.default_rng(0)
    X_full = rng.random((BATCH, N_NODES), np.float32)
    weights = rng.standard_normal((N_NODES, N_NODES), np.float32)
    bias = 0.001 * np.ones((N_NODES, 1), np.float32)
    edge_mask = rng.random((N_NODES, N_NODES)) < 0.002
    out = kernel(X_full, weights, bias, edge_mask)
    print("out", out.shape, out.dtype, out[:2, :4])


# revision 18
# speedup vs baseline: 1.3763x; 1.0372x over previous
"""Trainium2 Bass kernel for BioNet message-passing recurrence.

Reference computes 50 steps of Jacobi iteration  X <- mml(W @ X + X_bias)
with W (8192x8192 f32, masked) and X (8192x32), returning X.T (32, 8192).
The iteration is a contraction converging to a fixed point; ANY update
schedule converging to the same fixed point gives the same answer.  The
kernel uses block GAUSS-SEIDEL over two global half-blocks (nodes
[1024c,1024c+512) = half A, rest = half B): updating one half per
"half-step" using the freshest available other half converges in 6
sweeps (12 half-steps, verified to the bf16 noise floor) vs 11+ Jacobi
steps, and needs only ONE AllGather per half-step — which matters
because each collective costs ~6-8us of mostly-fixed protocol latency
on the single CC core.

Per-core layout (8 NeuronCores, tensor-parallel over W rows):
  - Each core holds rows [1024c, 1024c+1024) of W transposed in SBUF as
    bf16 (16.8 MB/core); DMAed in 8 K-chunks so step 2 starts early.
  - Half-step updating half U: 16 quads of out^T = X^T @ W^T with X
    (128,32) tiles stationary, 4-way column-tiled (tile_position) for
    ~4x PE throughput at batch=32.  Quads over U-parity K-columns use
    the 2-half-steps-old U state (long available, pre-run during the
    previous collective's flight); quads over the other parity wait for
    the just-gathered fresh half.
  - The 4 column-group partials land on partition groups 32j..32j+32 of
    PSUM; a small PE pass with selector S[p,b] = (p%32==b) fuses the
    4-way reduction with the (batch,node)->(node,batch) transpose; the
    PSUM->SBUF copy is split 4-ways to pipeline with the S-mms.
  - Activation uses  mml(z) = min(LeakyRelu_leak(z), 1-0.25/max(z,0.5))
    (algebraically exact); the LeakyRelu branch runs on the scalar
    engine in parallel with the saturation branch on DVE.
  - Queues: staging SBUF->DRAM and unload DMAs on sync (the unload's
    collective wait parks after the staging it follows, blocking
    nothing), collective triggers on gpsimd, LeakyRelu on the scalar
    queue, so no critical op ever queues behind a collective wait.
  - A dummy-matmul burst (same tile_position mode as the mains, so PE
    quad-concurrency streaks are not broken) spans the collective
    window: >3.4us of PE idle trips HAM clock-throttling, which halves
    the PE clock for ~10us.
"""

import os
import sys
import types

sys.path.insert(0, "/opt/trn_rl_repo")

import numpy as np
import ml_dtypes

import concourse.bass as bass
import concourse.mybir as mybir
import concourse.tile as tile
from concourse import bacc
import concourse.bass_utils as bass_utils
from concourse.bass import ts
from concourse.bass_utils import run_bass_kernel_spmd

N_NODES = 8192
N_CORES = 8
BATCH = 32
GS_HALF_STEPS = 12                  # 6 Gauss-Seidel sweeps (see module doc)
LEAK = 0.01
LOCAL = N_NODES // N_CORES          # 1024 rows per core
K_TILES = N_NODES // 128            # 64
LOCAL_TILES = LOCAL // 128          # 8
CHUNK_F = LOCAL_TILES * BATCH       # 256 free elems per activated chunk
HALF_F = CHUNK_F // 2               # 128
N_QUADS = K_TILES // 4              # 16
EVENS = list(range(0, N_QUADS, 2))  # K-columns of the A halves
ODDS = list(range(1, N_QUADS, 2))   # K-columns of the B halves
# SBUF W image K-tile order: even-quad tiles (A-half sources) first
K_ORDER = [4 * q + j for q in EVENS + ODDS for j in range(4)]
K_SLOT = {k: i for i, k in enumerate(K_ORDER)}

LAST_RESULTS = None  # BassKernelResults of the most recent run (for test.py)


def setup_tracing():
    """Register the axon NTFF profile hook; the container's antenv is a stub."""
    try:
        import antenv
        if "antenv.axon_hooks" not in sys.modules:
            mod = types.ModuleType("antenv.axon_hooks")
            mod._hook = None
            mod.set_axon_ntff_profile_hook = lambda h: setattr(mod, "_hook", h)
            mod.get_axon_ntff_profile_hook = lambda: mod._hook
            sys.modules["antenv.axon_hooks"] = mod
            antenv.axon_hooks = mod
            from trn_agent_boot.trn_boot import _ntff_profile_via_ctypes
            mod.set_axon_ntff_profile_hook(
                _ntff_profile_via_ctypes("/opt/axon/libaxon_pjrt.so")
            )
        bass_utils.upload_artifacts = lambda tmpdir: f"local://{tmpdir}"
    except Exception:
        pass


def build_nc():
    nc = bacc.Bacc(None, target_bir_lowering=False, num_devices=N_CORES)
    f32 = mybir.dt.float32
    bf16 = mybir.dt.bfloat16
    warm_per = int(os.environ.get("WARM_PER", "40"))

    # Per-core inputs (shapes identical on every core; contents sharded).
    wt = nc.dram_tensor("wt", [N_NODES, LOCAL], bf16, kind="ExternalInput")
    xb = nc.dram_tensor("xb", [128, CHUNK_F], f32, kind="ExternalInput")
    s_in = nc.dram_tensor("s_in", [128, BATCH], bf16, kind="ExternalInput")
    out = nc.dram_tensor("out", [128, CHUNK_F], f32, kind="ExternalOutput")

    with tile.TileContext(nc) as tc:
        with (
            tc.tile_pool(name="persist", bufs=1) as persist,
            tc.tile_pool(name="ys", bufs=2) as ys_pool,
            tc.tile_pool(name="chain", bufs=2) as chain,
            tc.tile_pool(name="stage", bufs=2) as stage_pool,
            tc.tile_pool(name="psum", bufs=2, space="PSUM") as psum_pool,
            tc.tile_pool(name="psumt", bufs=2, space="PSUM") as psumt_pool,
            tc.tile_pool(name="dram", bufs=2, space="DRAM") as dram,
        ):
            # ---- persistent SBUF tensors -------------------------------
            xb_sb = persist.tile([128, CHUNK_F], f32)
            nc.sync.dma_start(out=xb_sb, in_=xb[:])
            s_sb = persist.tile([128, BATCH], bf16)
            nc.scalar.dma_start(out=s_sb, in_=s_in[:])
            wt_sb = persist.tile([128, K_TILES, LOCAL], bf16)      # 128 KB/part
            # host stores K-tiles with even-quad (A-half source) tiles in
            # slots 0..31 and odd-quad tiles in 32..63 (K_SLOT): half-step
            # 2 only needs the first half of the image, so it starts after
            # ~half the 16.8 MB load
            wt_v = wt.rearrange("(t p) n -> p t n", p=128)
            for i in range(8):
                eng = nc.sync if i % 2 == 0 else nc.scalar
                eng.dma_start(
                    out=wt_sb[:, 8 * i : 8 * (i + 1), :],
                    in_=wt_v[:, 8 * i : 8 * (i + 1), :],
                )
            x_buf = persist.tile([128, K_TILES * BATCH], bf16)
            out_f32 = persist.tile([128, CHUNK_F], f32)

            def activation(z_src, to_bf, also_f32=None, width=HALF_F):
                """to_bf[:] = mml(z_src); optionally also f32 copy.

                mml(z) = min(LeakyRelu_leak(z), 1 - 0.25/max(z, 0.5))
                (exact for |z| < ~99, which holds here).  LeakyRelu branch
                on the scalar engine overlaps the DVE saturation chain.
                """
                lr_t = chain.tile([128, width], f32, tag="lr", name="lr_t")
                nc.scalar.activation(
                    lr_t, z_src, mybir.ActivationFunctionType.Lrelu,
                    alpha=LEAK,
                )
                m_t = chain.tile([128, width], f32, tag="m", name="m_t")
                nc.vector.tensor_scalar_max(m_t, z_src, 0.5)
                r_t = chain.tile([128, width], f32, tag="r", name="r_t")
                nc.vector.reciprocal_approx_fast(out=r_t, in_=m_t)
                s_t = chain.tile([128, width], f32, tag="s", name="s_t")
                nc.vector.tensor_scalar(
                    s_t, r_t, -0.25, 1.0,
                    mybir.AluOpType.mult, mybir.AluOpType.add,
                )
                nc.vector.tensor_tensor(to_bf, lr_t, s_t, mybir.AluOpType.min)
                if also_f32 is not None:
                    nc.vector.tensor_tensor(
                        also_f32, lr_t, s_t, mybir.AluOpType.min
                    )

            def tail_half(psum_hv, v, write_out):
                """Reduce+transpose (S-matrix PE pass), bias+activation for
                output half v; returns the staged bf16 (128, HALF_F) tile."""
                psum_t = psumt_pool.tile(
                    [128, HALF_F], mybir.dt.float32, tag="pt", name="psum_t"
                )
                for tt_ in range(4):
                    ysb = ys_pool.tile(
                        [128, 128], bf16, tag=f"ys{tt_}", name=f"ysb{tt_}"
                    )
                    nc.vector.tensor_copy(ysb, psum_hv[:, ts(tt_, 128)])
                    nc.tensor.matmul(
                        psum_t[:, ts(tt_, BATCH)],
                        ysb,
                        s_sb,
                        start=True,
                        stop=True,
                    )
                hs = ts(v, HALF_F)
                z_t = chain.tile([128, HALF_F], mybir.dt.float32,
                                 tag="z", name="z_t")
                nc.vector.tensor_tensor(
                    z_t, psum_t, xb_sb[:, hs], mybir.AluOpType.add
                )
                stage_v = stage_pool.tile(
                    [128, HALF_F], bf16, tag=f"st{v}", name=f"stage{v}"
                )
                activation(
                    z_t,
                    stage_v,
                    also_f32=out_f32[:, hs] if write_out else None,
                )
                return stage_v

            def broadcast_half(stage_v, v):
                """AllGather one staged half into x_buf's half-v columns."""
                ag_in = dram.tile([128, HALF_F], bf16, tag=f"agi{v}",
                                  name=f"ag_in{v}")
                nc.sync.dma_start(out=ag_in, in_=stage_v)
                ag_out = dram.tile(
                    [128 * N_CORES, HALF_F], bf16, addr_space="Shared",
                    tag=f"ago{v}", name=f"ag_out{v}",
                )
                nc.gpsimd.collective_compute(
                    "AllGather",
                    mybir.AluOpType.bypass,
                    replica_groups=[list(range(N_CORES))],
                    ins=[ag_in.opt()],
                    outs=[ag_out.opt()],
                )
                # strided unload: chunk c -> x_buf cols [256c+128v, +128);
                # split 2-way so the first fresh quads' data lands sooner
                dst_v = x_buf.rearrange("p (c f) -> p c f", c=N_CORES)[
                    :, :, HALF_F * v : HALF_F * (v + 1)
                ]
                src_v = ag_out.rearrange("(c p) f -> p c f", p=128)
                nc.sync.dma_start(out=dst_v[:, 0:3], in_=src_v[:, 0:3])
                nc.scalar.dma_start(out=dst_v[:, 3:6], in_=src_v[:, 3:6])
                nc.gpsimd.dma_start(out=dst_v[:, 6:8], in_=src_v[:, 6:8])

            def pe_warm():
                """Unpaced dummy matmuls filling most of the collective
                window so HAM never sees a >3.4us PE idle gap (which would
                halve the PE clock).  Sized to end ~3us before the fresh
                quads' data lands: the residual idle stays under the HAM
                threshold and an early collective is never blocked."""
                if warm_per <= 0:
                    return
                psum_w = psumt_pool.tile(
                    [128, 512], mybir.dt.float32, tag="pw", name="psum_w",
                    bufs=1,
                )
                # same 4-way tile_position mode as the mains: full-grid
                # matmuls here would break the quad-concurrency streaks
                # of neighbouring main matmuls (PE mode switch)
                for i in range(warm_per):
                    for j in range(4):
                        nc.tensor.matmul(
                            psum_w[32 * j : 32 * (j + 1), :],
                            s_sb,
                            wt_sb[:, j, 0:512],
                            start=True, stop=True, skip_group_check=True,
                            tile_position=(0, 32 * j),
                        )

            def mm_quads(h, psum_hv, quads, start, stop):
                for qi, q in enumerate(quads):
                    for j in range(4):
                        k = 4 * q + j
                        nc.tensor.matmul(
                            psum_hv[32 * j : 32 * (j + 1), :],
                            x_buf[:, ts(k, BATCH)],
                            wt_sb[:, K_SLOT[k], ts(h, 512)],
                            start=start and qi == 0,
                            stop=stop and qi == len(quads) - 1,
                            tile_position=(0, 32 * j),
                        )

            # ---- Gauss-Seidel half-steps -------------------------------
            # t odd: update half A (h=0);  t even: update half B (h=1).
            # Half-step t uses the fresh other half (gathered at t-1) and
            # its own 2-old half (gathered at t-2).
            for t in range(1, GS_HALF_STEPS + 1):
                h = (t + 1) % 2
                last = t == GS_HALF_STEPS
                write_out = t >= GS_HALF_STEPS - 1
                if t == 1:
                    # A(1) = mml(xb_A): state is zero, no matmuls
                    stage_v = stage_pool.tile(
                        [128, HALF_F], bf16, tag="st0", name="stage0"
                    )
                    activation(xb_sb[:, ts(0, HALF_F)], stage_v,
                               also_f32=out_f32[:, ts(0, HALF_F)]
                               if write_out else None)
                else:
                    stale = EVENS if h == 0 else ODDS
                    fresh = ODDS if h == 0 else EVENS
                    psum_hv = psum_pool.tile(
                        [128, 512], mybir.dt.float32, tag="pm", name="psum_m"
                    )
                    if t == 2:
                        # B(2) = mml(W_BA A(1) + xb_B): only A-columns
                        pe_warm()
                        mm_quads(1, psum_hv, EVENS, start=True, stop=True)
                    else:
                        mm_quads(h, psum_hv, stale, start=True, stop=False)
                        # paced warm fills the PE gap while the fresh
                        # half's collective is still in flight
                        pe_warm()
                        mm_quads(h, psum_hv, fresh, start=False, stop=True)
                    stage_v = tail_half(psum_hv, h, write_out)
                if last:
                    nc.sync.dma_start(out=out[:], in_=out_f32)
                else:
                    broadcast_half(stage_v, h)

    nc.compile()
    return nc


def _prepare_in_maps(X_full, weights, bias, edge_mask):
    W = np.where(edge_mask, weights, 0.0).astype(np.float32)
    Xb = X_full.astype(np.float32).T + bias.astype(np.float32)  # (n, B)
    S = np.zeros((128, BATCH), np.float32)
    S[np.arange(128), np.arange(128) % BATCH] = 1.0
    S = S.astype(ml_dtypes.bfloat16)
    in_maps = []
    for c in range(N_CORES):
        rows = slice(LOCAL * c, LOCAL * (c + 1))
        wt_c = np.ascontiguousarray(W[rows, :].T).astype(ml_dtypes.bfloat16)
        # group K-tiles so even-quad (A-half source) tiles come first,
        # matching K_SLOT in the kernel
        wt_c = (
            wt_c.reshape(K_TILES, 128, LOCAL)[K_ORDER]
            .reshape(N_NODES, LOCAL)
        )
        xb_c = (
            Xb[rows]                       # (1024, 32)
            .reshape(LOCAL_TILES, 128, BATCH)
            .transpose(1, 0, 2)
            .reshape(128, CHUNK_F)
            .copy()
        )
        in_maps.append({"wt": wt_c, "xb": xb_c, "s_in": S})
    return in_maps


def _reassemble(results):
    out = np.empty((BATCH, N_NODES), np.float32)
    for c in range(N_CORES):
        oc = np.asarray(results[c]["out"])  # (128, 256)
        chunk = (
            oc.reshape(128, LOCAL_TILES, BATCH)
            .transpose(1, 0, 2)
            .reshape(LOCAL, BATCH)
        )
        out[:, LOCAL * c : LOCAL * (c + 1)] = chunk.T
    return out


def kernel(X_full, weights, bias, edge_mask):
    global LAST_RESULTS
    setup_tracing()
    in_maps = _prepare_in_maps(X_full, weights, bias, edge_mask)
    nc = build_nc()
    res = run_bass_kernel_spmd(nc, in_maps, core_ids=list(range(N_CORES)))
    LAST_RESULTS = res
    return _reassemble(res.results)


if __name__ == "__main__":
    # quick self-run with random data
    rng = np.random.default_rng(0)
    X_full = rng.random((BATCH, N_NODES), np.float32)
    weights = rng.standard_normal((N_NODES, N_NODES), np.float32)
    bias = 0.001 * np.ones((N_NODES, 1), np.float32)
    edge_mask = rng.random((N_NODES, N_NODES)) < 0.002
    out = kernel(X_full, weights, bias, edge_mask)
    print("out", out.shape, out.dtype, out[:2, :4])
